# revision 1
# baseline (speedup 1.0000x reference)
"""Trainium2 Bass kernel for nn_CapsuleLayer (capsule layer: einsum + squash).

  u_hat = einsum('croi,bri->bcro', W[0], x)   # x:[256,1152,8] W:[1,10,1152,16,8]
  out   = squash(u_hat)                       # squash over last (o) axis

Strategy (8 NeuronCores, routes sharded 144/core, full batch per core):
  - Groups of 3 routes.  Per (group, batch-half) ONE psum bank holds both:
      u-MM:  stationary x^T block [32=(3 routes x 8 in + pad), 128 batch],
             moving block-diagonal W [32, 480] -> psum[:, 0:480]
      sq-MM: stationary xx pair-products [128=(3 x 36 pairs + pad), 128 batch],
             moving block-diagonal sym-Gram cols [128, 30] -> psum[:, 480:510]
    where xx[b,(i,j)] = x_i*x_j (i<=j) and Gsym[(i,j),c] = (2-delta_ij)*G[i,j]
    with G = W_cr^T W_cr, so sq-MM emits sq_norm[b, (r,c)] = ||u||^2 directly.
    The PE therefore replaces both the ACT square pass and the DVE group
    reduce of a conventional squash implementation.
  - squash scale s = sq/((1+sq)*sqrt(sq+1e-9)) = exp(0.5*ln(sq) - ln(1+sq))
    via ACT Ln/Exp (single activation-table set; Rsqrt/Reciprocal on ACT are
    banned for accuracy and DVE reciprocal is slow).
  - DVE does only the final broadcast multiply u * s straight out of PSUM
    into dense SBUF tiles; HWDGE DMAs ship contiguous 240KB blocks.
  - Matmuls run in float32r (single-pass reduced-precision fp32, 4x faster
    than fp32's two half-speed passes; measured end-to-end error ~5e-4
    scale-relative vs the fp32 reference).
"""

import sys

if "/opt/trn_rl_repo" not in sys.path:
    sys.path.insert(0, "/opt/trn_rl_repo")

from contextlib import ExitStack

import numpy as np

import concourse.bacc as bacc
import concourse.bass as bass
import concourse.mybir as mybir
import concourse.tile as tile
from concourse._compat import with_exitstack
from concourse.bass_utils import run_bass_kernel_spmd

# Problem shapes (hardcoded; harness provides full inputs)
B = 256          # batch
R = 1152         # num routes
C = 10           # num capsules
O = 16           # out channels
I = 8            # in channels
NCORES = 8
RL = R // NCORES                 # 144 routes per core
NG = RL // 3                     # 48 groups of 3 routes
NQ = NG // 4                     # 12 quad-blocks of 4 groups (row strips)
NPAIR = 36                       # i<=j pairs of 8 inputs
F32 = mybir.dt.float32
PAIRS = [(i, j) for i in range(I) for j in range(i, I)]


@with_exitstack
def _capsule_body(ctx: ExitStack, tc: "tile.TileContext",
                  out: bass.AP, xs: bass.AP, wm: bass.AP,
                  xxs: bass.AP, gs: bass.AP, reps: int = 1,
                  mode: str = "full"):
    nc = tc.nc

    if "fp32" in mode:
        mm_dt = F32
    else:
        mm_dt = mybir.dt.float32r
    # Optional: sq-path operands (xx pair products + gram cols) in bf16 —
    # halves the largest input tensor and enables FWL on the sq-matmul
    # stationary load; costs ~2x on the scale accuracy.
    sq_dt = mybir.dt.bfloat16 if "bxx" in mode else mm_dt

    singles = ctx.enter_context(tc.tile_pool(name="singles", bufs=1))
    wm_pool = ctx.enter_context(tc.tile_pool(name="wm", bufs=4))
    xx_pool = ctx.enter_context(tc.tile_pool(name="xx", bufs=4))
    psum_pool = ctx.enter_context(tc.tile_pool(name="psum", bufs=8, space="PSUM"))
    smalls = ctx.enter_context(tc.tile_pool(name="smalls", bufs=4))
    out_pool = ctx.enter_context(tc.tile_pool(name="outs", bufs=8))

    # Resident stationaries / gram columns — full-128-partition DMAs (32- or
    # 64-partition transfers run at a fraction of DMA port bandwidth).
    xs_sb = singles.tile([128, NQ * B], mm_dt)
    nc.gpsimd.dma_start(out=xs_sb[:], in_=xs.rearrange("p q b -> p (q b)"))
    gs_sb = singles.tile([128, NG * 30], sq_dt)
    nc.gpsimd.dma_start(out=gs_sb[:], in_=gs.rearrange("p g n -> p (g n)"))

    if reps > 1:
        # Timing-only variant: run the whole body `reps` times on-device so
        # wall-clock differences cancel host/axon overhead.
        loop_cm = tc.For_i(0, reps, 1)
        ctx.enter_context(loop_cm)

    # Per q: 4 groups stacked on the 4 row strips (partition blocks of 32);
    # iterate two half-blocks of 2 groups x 2 halves = 4 psum banks, so the
    # scale chain runs once per half-block on [128, 120] while psum bufs=8
    # double-buffers half-blocks.
    for q in range(NQ):
        wm_t = wm_pool.tile([128, 480], mm_dt)
        nc.gpsimd.dma_start(out=wm_t[:], in_=wm[q])
        xx_t = xx_pool.tile([128, 4 * B], sq_dt)
        nc.gpsimd.dma_start(out=xx_t[:], in_=xxs[q].rearrange("p k b -> p (k b)"))
        for half in range(2):
            quads = []
            sqb = smalls.tile([128, 120], F32, tag="sqb")
            # All four strip-tiled u-MMs first (different row groups -> the
            # PE reorder window can pull their weight loads ahead), then the
            # four full-array sq-MMs.
            for kk in range(2):
                k = 2 * half + kk
                g = 4 * q + k
                for h in range(2):
                    ps = psum_pool.tile([128, 512], F32, tag="ps")
                    nc.tensor.matmul(
                        ps[:, 0:480],
                        xs_sb[32 * k:32 * k + 32,
                              q * B + h * 128: q * B + h * 128 + 128],
                        wm_t[32 * k:32 * k + 32, :], start=True, stop=True,
                        tile_position=(32 * k, 0))
                    quads.append((ps, 2 * kk + h, g, h))
            for ps, j, g, h in quads:
                k = g - 4 * q
                nc.tensor.matmul(
                    ps[:, 480:510],
                    xx_t[:, k * B + h * 128: k * B + h * 128 + 128],
                    gs_sb[:, g * 30: g * 30 + 30], start=True, stop=True,
                    tile_position=(0, 0))
                if "nosquash" not in mode:
                    # sq: [128, (r3, c10)] -> copy into the block buffer (ACT
                    # is near PSUM and has slack; keeps DVE lean).
                    nc.scalar.copy(sqb[:, 30 * j: 30 * j + 30], ps[:, 480:510])

            if "nosquash" in mode:
                continue

            # Scale chain once per half-block: s = exp(0.5*ln(sq) - ln(1+sq))
            lnsq = smalls.tile([128, 120], F32, tag="lnsq")
            nc.scalar.activation(lnsq[:], sqb[:],
                                 mybir.ActivationFunctionType.Ln)
            ln1p = smalls.tile([128, 120], F32, tag="ln1p")
            nc.scalar.activation(ln1p[:], sqb[:],
                                 mybir.ActivationFunctionType.Ln, bias=1.0)
            w_t = smalls.tile([128, 120], F32, tag="w")
            nc.vector.scalar_tensor_tensor(
                out=w_t[:], in0=lnsq[:], scalar=0.5, in1=ln1p[:],
                op0=mybir.AluOpType.mult, op1=mybir.AluOpType.subtract)
            s_t = smalls.tile([128, 120], F32, tag="s")
            nc.scalar.activation(s_t[:], w_t[:],
                                 mybir.ActivationFunctionType.Exp)

            # Merged output tiles: the two kk-groups for one h are adjacent
            # in DRAM (g, g+1), so two muls share one [128, 960] tile and a
            # single 491KB store (96 -> 48 output DMAs).
            g0 = 4 * q + 2 * half
            ots = {}
            for ps, j, g, h in quads:
                if h not in ots:
                    ot_new = out_pool.tile([128, 960], F32, tag="ot")
                    ots[h] = ot_new
                u_ap = (ps[:, 0:480]
                        .rearrange("p (r c v) -> p r c v", r=3, c=C))
                s_b = (s_t[:, 30 * j: 30 * j + 30]
                       .rearrange("p (r c) -> p r c", r=3)
                       .unsqueeze(3).broadcast_to([128, 3, C, O]))
                kk = g - g0
                nc.vector.tensor_mul(
                    ots[h][:, 480 * kk: 480 * kk + 480]
                    .rearrange("p (r c v) -> p r c v", r=3, c=C), u_ap, s_b)
            if "noout" not in mode:
                for h in range(2):
                    nc.sync.dma_start(
                        out=out[h, g0:g0 + 2].rearrange("g p v -> p g v"),
                        in_=ots[h][:].rearrange("p (g v) -> p g v", g=2))


def build_bass(reps: int = 1, mode: str = "full"):
    # Bacc (not plain Bass): its compile() runs generate_event_semaphores,
    # which splits multi-semaphore waits — TPB instructions carry only one
    # wait slot in hardware — plus move_matmul_waits_to_ldweights etc.
    nc = bacc.Bacc("TRN2", target_bir_lowering=False, debug=False,
                   num_devices=NCORES)
    in_dt = F32 if "fp32" in mode else mybir.dt.float32r
    sq_in_dt = mybir.dt.bfloat16 if "bxx" in mode else in_dt
    xs = nc.dram_tensor("xs", [128, NQ, B], in_dt, kind="ExternalInput")
    wm = nc.dram_tensor("wm", [NQ, 128, 480], in_dt, kind="ExternalInput")
    xxs = nc.dram_tensor("xxs", [NQ, 128, 4, B], sq_in_dt, kind="ExternalInput")
    gs = nc.dram_tensor("gs", [128, NG, 30], sq_in_dt, kind="ExternalInput")
    out = nc.dram_tensor("out", [2, NG, 128, 480], F32, kind="ExternalOutput")
    with tile.TileContext(nc) as tc:
        _capsule_body(tc, out[:], xs[:], wm[:], xxs[:], gs[:],
                      reps=reps, mode=mode)

    # All ACT functions used here (Copy, Ln, Exp) coexist in the
    # natural_log_exp_and_others table set, but the stock table-load pass
    # assigns each function its *first* containing set, alternating sets and
    # inserting ~2.7us table loads throughout.  Strip our functions from all
    # other sets (keeping positional act_func_set ids intact) so resolution
    # lands on the one set and a single load is emitted.
    import types
    from concourse.hw_specs import get_activation_tables
    from concourse import bacc as _bacc_mod

    _PIN = "natural_log_exp_and_others"
    _FUNCS = {mybir.ActivationFunctionType.Square,
              mybir.ActivationFunctionType.Ln,
              mybir.ActivationFunctionType.Exp,
              mybir.ActivationFunctionType.Copy,
              mybir.ActivationFunctionType.Identity}

    def _one_set_table_loads(self):
        tables = [
            (k, (v if k == _PIN else (v - _FUNCS)))
            for k, v in get_activation_tables(self.m.arch).items()
        ]
        _bacc_mod._bass_rust.insert_act_table_loads(self, tables)

    nc.insert_act_table_loads = types.MethodType(_one_set_table_loads, nc)
    nc.compile()
    return nc


_NC = {}


def _get_nc(reps: int = 1, mode: str = "full"):
    key = (reps, mode)
    if key not in _NC:
        _NC[key] = build_bass(reps, mode)
    return _NC[key]


def _pack_inputs(x: np.ndarray, W: np.ndarray):
    """Build per-core xs [32,48,256], wm [48,32,480], xxs [48,128,256],
    gs [48,128,30]."""
    x = np.ascontiguousarray(x, dtype=np.float32)
    W0 = np.ascontiguousarray(W.reshape(C, R, O, I), dtype=np.float32)

    # x stationaries: [R, I, B] -> rows padded to 32, 4 groups stacked on the
    # 128 partitions (full-width DMA): [cores, 128=(k,row), NQ, B]
    xt = x.transpose(1, 2, 0)                        # [R, I, B]
    xs = np.zeros((NCORES, NG, 32, B), np.float32)
    xs[:, :, :24] = xt.reshape(NCORES, NG, 24, B)
    xs = xs.reshape(NCORES, NQ, 4, 32, B).transpose(0, 2, 3, 1, 4)
    xs = np.ascontiguousarray(xs.reshape(NCORES, 128, NQ, B))

    # W moving blocks, 4 groups stacked on partitions: [cores, NQ, 128, 480]
    Wt = W0.transpose(1, 3, 0, 2)                    # [R, I, C, O]
    Wt = Wt.reshape(NCORES, NG, 3, I, C * O)         # k,g,r,i,co
    wm = np.zeros((NCORES, NG, 32, 3, C * O), np.float32)
    for r in range(3):
        wm[:, :, r * I:(r + 1) * I, r] = Wt[:, :, r]
    wm = np.ascontiguousarray(wm.reshape(NCORES, NQ, 128, 480))

    # xx pair products: [B, R, 36] -> [cores, NQ, 4, (3*36 padded 128), B]
    ii = np.array([p[0] for p in PAIRS])
    jj = np.array([p[1] for p in PAIRS])
    xx = x[:, :, ii] * x[:, :, jj]                   # [B, R, 36]
    xxt = xx.transpose(1, 2, 0)                      # [R, 36, B]
    xxs = np.zeros((NCORES, NG, 128, B), np.float32)
    xxs[:, :, :108] = xxt.reshape(NCORES, NG, 108, B)
    xxs = np.ascontiguousarray(
        xxs.reshape(NCORES, NQ, 4, 128, B).transpose(0, 1, 3, 2, 4))

    # Gram columns: [cores, 48, 128, 30] block-diagonal over the 3 routes
    W64 = W0.astype(np.float64)
    G = np.einsum('croi,croj->crij', W64, W64)       # [C, R, I, I]
    Gsym = G[:, :, ii, jj] * np.where(ii == jj, 1.0, 2.0)   # [C, R, 36]
    Gt = Gsym.transpose(1, 2, 0).astype(np.float32)  # [R, 36, C]
    Gt = Gt.reshape(NCORES, NG, 3, NPAIR, C)
    gs = np.zeros((NCORES, NG, 128, 30), np.float32)
    for r in range(3):
        gs[:, :, r * NPAIR:(r + 1) * NPAIR, r * C:(r + 1) * C] = Gt[:, :, r]
    gs = np.ascontiguousarray(gs.transpose(0, 2, 1, 3))   # [cores, 128, 48, 30]
    return xs, wm, xxs, gs


def _unpack_outputs(results):
    """Per-core out [2, NG, 128, 480] -> full [B, C, R, O]."""
    full = np.empty((B, C, R, O), dtype=np.float32)
    for k in range(NCORES):
        ok = results[k]["out"].reshape(2, NG, 128, 3, C, O)
        # dims: h, g, p, r, c, o ; route_local = 3g + r
        fk = ok.transpose(0, 2, 4, 1, 3, 5).reshape(B, C, RL, O)
        full[:, :, k * RL:(k + 1) * RL, :] = fk
    return full


def run_packed(packed, reps: int = 1, mode: str = "full"):
    xs, wm, xxs, gs = packed
    if "bxx" in mode:
        import ml_dtypes
        xxs = xxs.astype(ml_dtypes.bfloat16)
        gs = gs.astype(ml_dtypes.bfloat16)
    nc = _get_nc(reps, mode)
    in_maps = [{"xs": xs[k], "wm": wm[k], "xxs": xxs[k], "gs": gs[k]}
               for k in range(NCORES)]
    return run_bass_kernel_spmd(nc, in_maps, list(range(NCORES)))


def kernel(x: np.ndarray, W: np.ndarray, **_ignored):
    x = np.asarray(x, dtype=np.float32)
    W = np.asarray(W, dtype=np.float32)
    assert x.shape == (B, R, I), x.shape
    packed = _pack_inputs(x, W)
    res = run_packed(packed)
    return _unpack_outputs(res.results)



# revision 7
# speedup vs baseline: 1.3179x; 1.3179x over previous
"""Trainium2 Bass kernel for nn_CapsuleLayer (capsule layer: einsum + squash).

  u_hat = einsum('croi,bri->bcro', W[0], x)   # x:[256,1152,8] W:[1,10,1152,16,8]
  out   = squash(u_hat)                       # squash over last (o) axis

Strategy (8 NeuronCores, routes sharded 144/core, full batch per core):
  - Groups of 3 routes.  Per (group, batch-half) ONE psum bank holds both:
      u-MM:  stationary x^T block [32=(3 routes x 8 in + pad), 128 batch],
             moving block-diagonal W [32, 480] -> psum[:, 0:480]
      sq-MM: stationary xx pair-products [128=(3 x 36 pairs + pad), 128 batch],
             moving block-diagonal sym-Gram cols [128, 30] -> psum[:, 480:510]
    where xx[b,(i,j)] = x_i*x_j (i<=j) and Gsym[(i,j),c] = (2-delta_ij)*G[i,j]
    with G = W_cr^T W_cr, so sq-MM emits sq_norm[b, (r,c)] = ||u||^2 directly.
    The PE therefore replaces both the ACT square pass and the DVE group
    reduce of a conventional squash implementation.
  - squash scale s = sq/((1+sq)*sqrt(sq+1e-9)) = exp(0.5*ln(sq) - ln(1+sq))
    via ACT Ln/Exp (single activation-table set; Rsqrt/Reciprocal on ACT are
    banned for accuracy and DVE reciprocal is slow).
  - DVE does only the final broadcast multiply u * s straight out of PSUM
    into dense SBUF tiles; HWDGE DMAs ship contiguous 240KB blocks.
  - Matmuls run in float32r (single-pass reduced-precision fp32, 4x faster
    than fp32's two half-speed passes; measured end-to-end error ~5e-4
    scale-relative vs the fp32 reference).
"""

import sys

if "/opt/trn_rl_repo" not in sys.path:
    sys.path.insert(0, "/opt/trn_rl_repo")

from contextlib import ExitStack

import numpy as np

import concourse.bacc as bacc
import concourse.bass as bass
import concourse.mybir as mybir
import concourse.tile as tile
from concourse._compat import with_exitstack
from concourse.bass_utils import run_bass_kernel_spmd

# Problem shapes (hardcoded; harness provides full inputs)
B = 256          # batch
R = 1152         # num routes
C = 10           # num capsules
O = 16           # out channels
I = 8            # in channels
NCORES = 8
RL = R // NCORES                 # 144 routes per core
NG = RL // 3                     # 48 groups of 3 routes
NQ = NG // 4                     # 12 quad-blocks of 4 groups (row strips)
NPAIR = 36                       # i<=j pairs of 8 inputs
F32 = mybir.dt.float32
PAIRS = [(i, j) for i in range(I) for j in range(i, I)]


@with_exitstack
def _capsule_body(ctx: ExitStack, tc: "tile.TileContext",
                  out: bass.AP, xs: bass.AP, wm: bass.AP,
                  xxs: bass.AP, gs: bass.AP, reps: int = 1,
                  mode: str = "full"):
    nc = tc.nc

    if "fp32" in mode:
        mm_dt = F32
    elif "u16" in mode:
        # fp16 u-path operands: halves the wm stream and keeps matmul at
        # 1 cycle/row; fp16 mantissa (2^-11) keeps error ~1e-3.
        mm_dt = mybir.dt.float16
    else:
        mm_dt = mybir.dt.float32r
    # Optional: sq-path operands (xx pair products + gram cols) in 16-bit —
    # halves the largest input tensor, enables FWL on the sq-matmul
    # stationary load, and lifts the f32r small-moving (30<256) 4-cyc/row
    # penalty on the sq matmul.
    if "q16" in mode:
        sq_dt = mybir.dt.float16
    elif "bxx" in mode:
        sq_dt = mybir.dt.bfloat16
    else:
        sq_dt = mm_dt
    # fp16 output tiles halve the dominant (23.6MB/core) output stream;
    # host upcasts.  |out| <= 1 so fp16 abs error <= 2^-11.
    out_dt = mybir.dt.float16 if "o16" in mode else F32

    singles = ctx.enter_context(tc.tile_pool(name="singles", bufs=1))
    wm_pool = ctx.enter_context(tc.tile_pool(name="wm", bufs=4))
    xx_pool = ctx.enter_context(tc.tile_pool(name="xx", bufs=4))
    psum_pool = ctx.enter_context(tc.tile_pool(name="psum", bufs=8, space="PSUM"))
    smalls = ctx.enter_context(tc.tile_pool(name="smalls", bufs=4))
    out_pool = ctx.enter_context(tc.tile_pool(name="outs", bufs=8))

    # Resident stationaries / gram columns — full-128-partition DMAs (32- or
    # 64-partition transfers run at a fraction of DMA port bandwidth).
    xs_sb = singles.tile([128, NQ * B], mm_dt)
    nc.gpsimd.dma_start(out=xs_sb[:], in_=xs.rearrange("p q b -> p (q b)"))
    gs_sb = singles.tile([128, NG * 30], sq_dt)
    nc.gpsimd.dma_start(out=gs_sb[:], in_=gs.rearrange("p g n -> p (g n)"))

    if reps > 1:
        # Timing-only variant: run the whole body `reps` times on-device so
        # wall-clock differences cancel host/axon overhead.
        loop_cm = tc.For_i(0, reps, 1)
        ctx.enter_context(loop_cm)

    # Per q: 4 groups stacked on the 4 row strips (partition blocks of 32);
    # iterate two half-blocks of 2 groups x 2 halves = 4 psum banks, so the
    # scale chain runs once per half-block on [128, 120] while psum bufs=8
    # double-buffers half-blocks.
    for q in range(NQ):
        wm_t = wm_pool.tile([128, 480], mm_dt)
        nc.gpsimd.dma_start(out=wm_t[:], in_=wm[q])
        xx_t = xx_pool.tile([128, 4 * B], sq_dt)
        nc.gpsimd.dma_start(out=xx_t[:], in_=xxs[q].rearrange("p k b -> p (k b)"))
        for half in range(2):
            quads = []
            sqb = smalls.tile([128, 120], F32, tag="sqb")
            # All four strip-tiled u-MMs first (different row groups -> the
            # PE reorder window can pull their weight loads ahead), then the
            # four full-array sq-MMs.
            for kk in range(2):
                k = 2 * half + kk
                g = 4 * q + k
                for h in range(2):
                    ps = psum_pool.tile([128, 512], F32, tag="ps")
                    nc.tensor.matmul(
                        ps[:, 0:480],
                        xs_sb[32 * k:32 * k + 32,
                              q * B + h * 128: q * B + h * 128 + 128],
                        wm_t[32 * k:32 * k + 32, :], start=True, stop=True,
                        tile_position=(32 * k, 0))
                    quads.append((ps, 2 * kk + h, g, h))
            for ps, j, g, h in quads:
                k = g - 4 * q
                nc.tensor.matmul(
                    ps[:, 480:510],
                    xx_t[:, k * B + h * 128: k * B + h * 128 + 128],
                    gs_sb[:, g * 30: g * 30 + 30], start=True, stop=True,
                    tile_position=(0, 0))
                if "nosquash" not in mode:
                    # sq: [128, (r3, c10)] -> copy into the block buffer (ACT
                    # is near PSUM and has slack; keeps DVE lean).
                    nc.scalar.copy(sqb[:, 30 * j: 30 * j + 30], ps[:, 480:510])

            if "nosquash" in mode:
                continue

            # Scale chain once per half-block: s = exp(0.5*ln(sq) - ln(1+sq))
            lnsq = smalls.tile([128, 120], F32, tag="lnsq")
            nc.scalar.activation(lnsq[:], sqb[:],
                                 mybir.ActivationFunctionType.Ln)
            ln1p = smalls.tile([128, 120], F32, tag="ln1p")
            nc.scalar.activation(ln1p[:], sqb[:],
                                 mybir.ActivationFunctionType.Ln, bias=1.0)
            w_t = smalls.tile([128, 120], F32, tag="w")
            nc.vector.scalar_tensor_tensor(
                out=w_t[:], in0=lnsq[:], scalar=0.5, in1=ln1p[:],
                op0=mybir.AluOpType.mult, op1=mybir.AluOpType.subtract)
            s_t = smalls.tile([128, 120], F32, tag="s")
            nc.scalar.activation(s_t[:], w_t[:],
                                 mybir.ActivationFunctionType.Exp)

            # Merged output tiles: the two kk-groups for one h are adjacent
            # in DRAM (g, g+1), so two muls share one [128, 960] tile and a
            # single 491KB store (96 -> 48 output DMAs).
            g0 = 4 * q + 2 * half
            ots = {}
            for ps, j, g, h in quads:
                if h not in ots:
                    ot_new = out_pool.tile([128, 960], out_dt, tag="ot")
                    ots[h] = ot_new
                u_ap = (ps[:, 0:480]
                        .rearrange("p (r c v) -> p r c v", r=3, c=C))
                s_b = (s_t[:, 30 * j: 30 * j + 30]
                       .rearrange("p (r c) -> p r c", r=3)
                       .unsqueeze(3).broadcast_to([128, 3, C, O]))
                kk = g - g0
                nc.vector.tensor_mul(
                    ots[h][:, 480 * kk: 480 * kk + 480]
                    .rearrange("p (r c v) -> p r c v", r=3, c=C), u_ap, s_b)
            if "noout" not in mode:
                # out[h, q, half] is one contiguous [128, 960] block: a
                # single per-partition segment per DMA (vs 2 x 1920B with
                # the per-group layout).
                for h in range(2):
                    nc.sync.dma_start(out=out[h, q, half], in_=ots[h][:])


def build_bass(reps: int = 1, mode: str = "full"):
    # Bacc (not plain Bass): its compile() runs generate_event_semaphores,
    # which splits multi-semaphore waits — TPB instructions carry only one
    # wait slot in hardware — plus move_matmul_waits_to_ldweights etc.
    nc = bacc.Bacc("TRN2", target_bir_lowering=False, debug=False,
                   num_devices=NCORES)
    if "fp32" in mode:
        in_dt = F32
    elif "u16" in mode:
        in_dt = mybir.dt.float16
    else:
        in_dt = mybir.dt.float32r
    if "q16" in mode:
        sq_in_dt = mybir.dt.float16
    elif "bxx" in mode:
        sq_in_dt = mybir.dt.bfloat16
    else:
        sq_in_dt = in_dt
    out_dt = mybir.dt.float16 if "o16" in mode else F32
    xs = nc.dram_tensor("xs", [128, NQ, B], in_dt, kind="ExternalInput")
    wm = nc.dram_tensor("wm", [NQ, 128, 480], in_dt, kind="ExternalInput")
    xxs = nc.dram_tensor("xxs", [NQ, 128, 4, B], sq_in_dt, kind="ExternalInput")
    gs = nc.dram_tensor("gs", [128, NG, 30], sq_in_dt, kind="ExternalInput")
    out = nc.dram_tensor("out", [2, NQ, 2, 128, 960], out_dt,
                         kind="ExternalOutput")
    with tile.TileContext(nc) as tc:
        _capsule_body(tc, out[:], xs[:], wm[:], xxs[:], gs[:],
                      reps=reps, mode=mode)

    # All ACT functions used here (Copy, Ln, Exp) coexist in the
    # natural_log_exp_and_others table set, but the stock table-load pass
    # assigns each function its *first* containing set, alternating sets and
    # inserting ~2.7us table loads throughout.  Strip our functions from all
    # other sets (keeping positional act_func_set ids intact) so resolution
    # lands on the one set and a single load is emitted.
    import types
    from concourse.hw_specs import get_activation_tables
    from concourse import bacc as _bacc_mod

    _PIN = "natural_log_exp_and_others"
    _FUNCS = {mybir.ActivationFunctionType.Square,
              mybir.ActivationFunctionType.Ln,
              mybir.ActivationFunctionType.Exp,
              mybir.ActivationFunctionType.Copy,
              mybir.ActivationFunctionType.Identity}

    def _one_set_table_loads(self):
        tables = [
            (k, (v if k == _PIN else (v - _FUNCS)))
            for k, v in get_activation_tables(self.m.arch).items()
        ]
        _bacc_mod._bass_rust.insert_act_table_loads(self, tables)

    nc.insert_act_table_loads = types.MethodType(_one_set_table_loads, nc)
    nc.compile()
    return nc


_NC = {}


def _get_nc(reps: int = 1, mode: str = "full"):
    key = (reps, mode)
    if key not in _NC:
        _NC[key] = build_bass(reps, mode)
    return _NC[key]


def _pack_inputs(x: np.ndarray, W: np.ndarray):
    """Build per-core xs [32,48,256], wm [48,32,480], xxs [48,128,256],
    gs [48,128,30]."""
    x = np.ascontiguousarray(x, dtype=np.float32)
    W0 = np.ascontiguousarray(W.reshape(C, R, O, I), dtype=np.float32)

    # x stationaries: [R, I, B] -> rows padded to 32, 4 groups stacked on the
    # 128 partitions (full-width DMA): [cores, 128=(k,row), NQ, B]
    xt = x.transpose(1, 2, 0)                        # [R, I, B]
    xs = np.zeros((NCORES, NG, 32, B), np.float32)
    xs[:, :, :24] = xt.reshape(NCORES, NG, 24, B)
    xs = xs.reshape(NCORES, NQ, 4, 32, B).transpose(0, 2, 3, 1, 4)
    xs = np.ascontiguousarray(xs.reshape(NCORES, 128, NQ, B))

    # W moving blocks, 4 groups stacked on partitions: [cores, NQ, 128, 480]
    Wt = W0.transpose(1, 3, 0, 2)                    # [R, I, C, O]
    Wt = Wt.reshape(NCORES, NG, 3, I, C * O)         # k,g,r,i,co
    wm = np.zeros((NCORES, NG, 32, 3, C * O), np.float32)
    for r in range(3):
        wm[:, :, r * I:(r + 1) * I, r] = Wt[:, :, r]
    wm = np.ascontiguousarray(wm.reshape(NCORES, NQ, 128, 480))

    # xx pair products: [B, R, 36] -> [cores, NQ, 4, (3*36 padded 128), B]
    ii = np.array([p[0] for p in PAIRS])
    jj = np.array([p[1] for p in PAIRS])
    xx = x[:, :, ii] * x[:, :, jj]                   # [B, R, 36]
    xxt = xx.transpose(1, 2, 0)                      # [R, 36, B]
    xxs = np.zeros((NCORES, NG, 128, B), np.float32)
    xxs[:, :, :108] = xxt.reshape(NCORES, NG, 108, B)
    xxs = np.ascontiguousarray(
        xxs.reshape(NCORES, NQ, 4, 128, B).transpose(0, 1, 3, 2, 4))

    # Gram columns: [cores, 48, 128, 30] block-diagonal over the 3 routes
    W64 = W0.astype(np.float64)
    G = np.einsum('croi,croj->crij', W64, W64)       # [C, R, I, I]
    Gsym = G[:, :, ii, jj] * np.where(ii == jj, 1.0, 2.0)   # [C, R, 36]
    Gt = Gsym.transpose(1, 2, 0).astype(np.float32)  # [R, 36, C]
    Gt = Gt.reshape(NCORES, NG, 3, NPAIR, C)
    gs = np.zeros((NCORES, NG, 128, 30), np.float32)
    for r in range(3):
        gs[:, :, r * NPAIR:(r + 1) * NPAIR, r * C:(r + 1) * C] = Gt[:, :, r]
    gs = np.ascontiguousarray(gs.transpose(0, 2, 1, 3))   # [cores, 128, 48, 30]
    return xs, wm, xxs, gs


def _unpack_outputs(results):
    """Per-core out [2, NQ, 2, 128, 960] -> full [B, C, R, O]."""
    full = np.empty((B, C, R, O), dtype=np.float32)
    for k in range(NCORES):
        ok = np.asarray(results[k]["out"], dtype=np.float32)
        # dims: hb, q, half, p, kk, r, c, o ;
        # route_local = 3*(4q + 2*half + kk) + r ; b = 128*hb + p
        ok = ok.reshape(2, NQ, 2, 128, 2, 3, C, O)
        fk = ok.transpose(0, 3, 6, 1, 2, 4, 5, 7).reshape(B, C, RL, O)
        full[:, :, k * RL:(k + 1) * RL, :] = fk
    return full


DEFAULT_MODE = "full+o16+u16+q16"


def _cast_packed(packed, mode: str):
    xs, wm, xxs, gs = packed
    if "u16" in mode:
        xs = xs.astype(np.float16)
        wm = wm.astype(np.float16)
    if "q16" in mode:
        xxs = xxs.astype(np.float16)
        gs = gs.astype(np.float16)
    elif "bxx" in mode:
        import ml_dtypes
        xxs = xxs.astype(ml_dtypes.bfloat16)
        gs = gs.astype(ml_dtypes.bfloat16)
    return xs, wm, xxs, gs


def run_packed(packed, reps: int = 1, mode: str = DEFAULT_MODE):
    xs, wm, xxs, gs = _cast_packed(packed, mode)
    nc = _get_nc(reps, mode)
    in_maps = [{"xs": xs[k], "wm": wm[k], "xxs": xxs[k], "gs": gs[k]}
               for k in range(NCORES)]
    return run_bass_kernel_spmd(nc, in_maps, list(range(NCORES)))


def kernel(x: np.ndarray, W: np.ndarray, **_ignored):
    x = np.asarray(x, dtype=np.float32)
    W = np.asarray(W, dtype=np.float32)
    assert x.shape == (B, R, I), x.shape
    packed = _pack_inputs(x, W)
    res = run_packed(packed)
    return _unpack_outputs(res.results)



# revision 27
# speedup vs baseline: 2.1578x; 1.6373x over previous
"""Trainium2 Bass kernel for nn_CapsuleLayer (capsule layer: einsum + squash).

  u_hat = einsum('croi,bri->bcro', W[0], x)   # x:[256,1152,8] W:[1,10,1152,16,8]
  out   = squash(u_hat)                       # squash over last (o) axis

Strategy (8 NeuronCores, routes sharded 144/core, full batch per core):
  - Groups of 3 routes.  Per (group, batch-half) ONE psum bank holds both:
      u-MM:  stationary x^T block [32=(3 routes x 8 in + pad), 128 batch],
             moving block-diagonal W [32, 480] -> psum[:, 0:480]
      sq-MM: stationary xx pair-products [128=(3 x 36 pairs + pad), 128 batch],
             moving block-diagonal sym-Gram cols [128, 30] -> psum[:, 480:510]
    where xx[b,(i,j)] = x_i*x_j (i<=j) and Gsym[(i,j),c] = (2-delta_ij)*G[i,j]
    with G = W_cr^T W_cr, so sq-MM emits sq_norm[b, (r,c)] = ||u||^2 directly.
    The PE therefore replaces both the ACT square pass and the DVE group
    reduce of a conventional squash implementation.
  - squash scale s = sq/((1+sq)*sqrt(sq+1e-9)) = exp(0.5*ln(sq) - ln(1+sq))
    via ACT Ln/Exp (single activation-table set; Rsqrt/Reciprocal on ACT are
    banned for accuracy and DVE reciprocal is slow).
  - DVE does only the final broadcast multiply u * s straight out of PSUM
    into dense SBUF tiles; HWDGE DMAs ship contiguous 240KB blocks.
  - Matmuls run in float32r (single-pass reduced-precision fp32, 4x faster
    than fp32's two half-speed passes; measured end-to-end error ~5e-4
    scale-relative vs the fp32 reference).
"""

import sys

if "/opt/trn_rl_repo" not in sys.path:
    sys.path.insert(0, "/opt/trn_rl_repo")

from contextlib import ExitStack

import numpy as np

import concourse.bacc as bacc
import concourse.bass as bass
import concourse.mybir as mybir
import concourse.tile as tile
from concourse._compat import with_exitstack
from concourse.bass_utils import run_bass_kernel_spmd

# Problem shapes (hardcoded; harness provides full inputs)
B = 256          # batch
R = 1152         # num routes
C = 10           # num capsules
O = 16           # out channels
I = 8            # in channels
NCORES = 8
RL = R // NCORES                 # 144 routes per core
NG = RL // 3                     # 48 groups of 3 routes
NQ = NG // 4                     # 12 quad-blocks of 4 groups (row strips)
NPAIR = 36                       # i<=j pairs of 8 inputs
F32 = mybir.dt.float32
PAIRS = [(i, j) for i in range(I) for j in range(i, I)]


@with_exitstack
def _capsule_body(ctx: ExitStack, tc: "tile.TileContext",
                  out: bass.AP, xs: bass.AP, wm: bass.AP,
                  xxs: bass.AP, gs: bass.AP, reps: int = 1,
                  mode: str = "full"):
    nc = tc.nc

    if "fp32" in mode:
        mm_dt = F32
    elif "u16" in mode:
        # fp16 u-path operands: halves the wm stream and keeps matmul at
        # 1 cycle/row; fp16 mantissa (2^-11) keeps error ~1e-3.
        mm_dt = mybir.dt.float16
    else:
        mm_dt = mybir.dt.float32r
    # Optional: sq-path operands (xx pair products + gram cols) in 16-bit —
    # halves the largest input tensor, enables FWL on the sq-matmul
    # stationary load, and lifts the f32r small-moving (30<256) 4-cyc/row
    # penalty on the sq matmul.
    if "q16" in mode:
        sq_dt = mybir.dt.float16
    elif "bxx" in mode:
        sq_dt = mybir.dt.bfloat16
    else:
        sq_dt = mm_dt
    # fp16 output tiles halve the dominant (23.6MB/core) output stream;
    # host upcasts.  |out| <= 1 so fp16 abs error <= 2^-11.
    out_dt = mybir.dt.float16 if "o16" in mode else F32

    import re

    # '+dN': of the 96 u-mul bank-units per rep, N are multiplied by DVE
    # straight out of PSUM (merged per-half-block instructions); the rest
    # are staged psum->SBUF by ACT and multiplied by Pool (gpsimd), which
    # cannot touch PSUM on TRN2.  Balances DVE/ACT/Pool busy time.
    ndve = re.search(r"\+d(\d+)", mode)
    ndve = int(ndve.group(1)) if ndve else 60
    assert 0 <= ndve <= 96

    singles = ctx.enter_context(tc.tile_pool(name="singles", bufs=1))
    wm_pool = ctx.enter_context(tc.tile_pool(name="wm", bufs=NQ))
    xx_pool = ctx.enter_context(tc.tile_pool(name="xx", bufs=NQ))
    psum_pool = ctx.enter_context(tc.tile_pool(name="psum", bufs=3, space="PSUM"))
    sq_pool = ctx.enter_context(tc.tile_pool(name="sqp", bufs=2, space="PSUM"))
    smalls = ctx.enter_context(tc.tile_pool(name="smalls", bufs=4))
    stage_pool = ctx.enter_context(tc.tile_pool(name="stage", bufs=4))
    out_pool = ctx.enter_context(tc.tile_pool(name="outs", bufs=8))

    # Resident stationaries / gram columns — full-128-partition DMAs (32- or
    # 64-partition transfers run at a fraction of DMA port bandwidth).
    xs_sb = singles.tile([128, NQ * B], mm_dt)
    nc.gpsimd.dma_start(out=xs_sb[:], in_=xs.rearrange("p q b -> p (q b)"))
    gs_sb = singles.tile([128, NG * 30], sq_dt)
    nc.gpsimd.dma_start(out=gs_sb[:], in_=gs.rearrange("p g n -> p (g n)"))
    # wm + xx resident too (5.8MB fp16 total): all input streaming happens
    # once, outside the rep loop; the per-rep loop then only writes output,
    # and the Pool engine is free to take a share of the u-muls.
    wm_sb = []
    xx_sb = []
    for q in range(NQ):
        wt = wm_pool.tile([128, 480], mm_dt)
        nc.gpsimd.dma_start(out=wt[:], in_=wm[q])
        wm_sb.append(wt)
        xt = xx_pool.tile([128, 4 * B], sq_dt)
        nc.gpsimd.dma_start(out=xt[:], in_=xxs[q].rearrange("p k b -> p (k b)"))
        xx_sb.append(xt)

    # Software-pipelined schedule over 48 "blocks" (one route-group g =
    # 4q + 2*half + kk each, both batch halves h).  Block n's u lives in a
    # 2-bank psum tile [128,1024] (bank = h), pool bufs=3 -> pipeline
    # depth 3.  sq is DECOUPLED from the u banks: windows of 4 blocks
    # write their 8 sq results into dedicated [128,240] psum tiles
    # (2 bufs), emitted a full window ahead, so the ACT scale chains run
    # with multi-block lead and the u tiles free as soon as the muls/
    # copies drain them.  Block 0's window+chain come from a one-time
    # prologue; the loop's last window/chain re-computes them for the
    # next rep (same pool slots: allocation counts per rep are multiples
    # of bufs).
    nosq = "nosquash" in mode
    NBL = 4 * NQ                      # 48 blocks; block n == group g=n
    s_tiles = {}

    def sq_window(w, sqt):
        # sq matmuls for blocks 4w..4w+3 into sqt cols [(m, h, 30)].
        for m in range(4):
            g = 4 * w + m
            q, k = g // 4, g % 4
            for h in range(2):
                nc.tensor.matmul(
                    sqt[:, 60 * m + 30 * h: 60 * m + 30 * h + 30],
                    xx_sb[q][:, k * B + h * 128: k * B + h * 128 + 128],
                    gs_sb[:, g * 30: g * 30 + 30], start=True, stop=True,
                    tile_position=(0, 0))

    def chain(p, sqt):
        # Scale chain for block pair p (blocks 2p, 2p+1) reading its half
        # of sq window p//2: s = exp(0.5*ln(sq) - ln(1+sq)).
        sq_ap = sqt[:, 120 * (p % 2): 120 * (p % 2) + 120]
        lnsq = smalls.tile([128, 120], F32, tag="lnsq")
        nc.scalar.activation(lnsq[:], sq_ap,
                             mybir.ActivationFunctionType.Ln)
        ln1p = smalls.tile([128, 120], F32, tag="ln1p")
        nc.scalar.activation(ln1p[:], sq_ap,
                             mybir.ActivationFunctionType.Ln, bias=1.0)
        w_t = smalls.tile([128, 120], F32, tag="w")
        # '+sp': combine on Pool instead of DVE.
        eng = nc.gpsimd if "+sp" in mode else nc.vector
        eng.scalar_tensor_tensor(
            out=w_t[:], in0=lnsq[:], scalar=0.5, in1=ln1p[:],
            op0=mybir.AluOpType.mult, op1=mybir.AluOpType.subtract)
        s_t = smalls.tile([128, 120], F32, tag="s")
        nc.scalar.activation(s_t[:], w_t[:],
                             mybir.ActivationFunctionType.Exp)
        return s_t

    if not nosq:
        # Prologue: sq window 0 + chain for pair 0.
        sqt0 = sq_pool.tile([128, 240], F32, tag="sq")
        sq_window(0, sqt0)
        s_tiles[0] = chain(0, sqt0)
        sq_cur = sqt0

    if reps > 1:
        # Timing-only variant: run the whole body `reps` times on-device so
        # wall-clock differences cancel host/axon overhead.
        loop_cm = tc.For_i(0, reps, 1)
        ctx.enter_context(loop_cm)

    for n in range(NBL):
        g = n
        q, k = g // 4, g % 4
        if not nosq and n % 4 == 0:
            # sq matmuls for the NEXT window (blocks n+4..n+7).
            sq_next = sq_pool.tile([128, 240], F32, tag="sq")
            sq_window((n // 4 + 1) % NQ, sq_next)
        if not nosq and n % 2 == 0:
            # chain for the NEXT pair; pair p reads window p//2, which was
            # emitted >= 2 blocks ago.
            p = (n // 2 + 1) % (NBL // 2)
            src = sq_cur if (p % 2) else sq_next
            s_tiles[p] = chain(p, src)
            if n % 4 == 2:
                sq_cur = sq_next

        ps = psum_pool.tile([128, 1024], F32, tag="ps")
        psb = ps[:].rearrange("p (b w) -> p b w", w=512)
        for h in range(2):
            nc.tensor.matmul(
                ps[:, 512 * h: 512 * h + 480],
                xs_sb[32 * k:32 * k + 32,
                      q * B + h * 128: q * B + h * 128 + 128],
                wm_sb[q][32 * k:32 * k + 32, :], start=True, stop=True,
                tile_position=(32 * k, 0))
        if nosq:
            continue
        s_t = s_tiles[n // 2]
        e = n % 2                      # which half of the pair's s

        # Out tile per block [128, 960] = (h, rc, v); nd of the 2 banks
        # multiplied by DVE straight from psum, the rest staged to SBUF
        # by ACT and multiplied by Pool (which cannot touch PSUM).
        # Bresenham over 96 bank-units hits the '+dN' global DVE share.
        ot = out_pool.tile([128, 960], out_dt, tag="ot")
        nd = ((n + 1) * ndve * 2 // 96) - (n * ndve * 2 // 96)
        nd = max(0, min(2, nd))

        def u_ap(b0, nb):
            return (psb[:, b0:b0 + nb, 0:480]
                    .rearrange("p b (rc v) -> p b rc v", v=O))

        def s_ap(b0, nb):
            return (s_t[:, 60 * e + 30 * b0: 60 * e + 30 * (b0 + nb)]
                    .rearrange("p (b rc) -> p b rc", b=nb)
                    .unsqueeze(3).broadcast_to([128, nb, 30, O]))

        def o_ap(b0, nb):
            return (ot[:, 480 * b0: 480 * (b0 + nb)]
                    .rearrange("p (b rc v) -> p b rc v", b=nb, v=O))

        if nd > 0:
            nc.vector.tensor_mul(o_ap(0, nd), u_ap(0, nd), s_ap(0, nd))
        nb = 2 - nd
        if nb > 0:
            st = stage_pool.tile([128, 960], F32, tag="stg")
            nc.scalar.copy(
                st[:, 0:480 * nb].rearrange("p (b v) -> p b v", b=nb),
                psb[:, nd:2, 0:480])
            st_ap = (st[:, 0:480 * nb]
                     .rearrange("p (b rc v) -> p b rc v", b=nb, v=O))
            nc.gpsimd.tensor_mul(o_ap(nd, nb), st_ap, s_ap(nd, nb))

        if "noout" not in mode:
            # One DMA per block: a single contiguous 1920B (fp16) segment
            # per partition.
            nc.sync.dma_start(out=out[q, k // 2, k % 2], in_=ot[:])


def build_bass(reps: int = 1, mode: str = "full"):
    # Bacc (not plain Bass): its compile() runs generate_event_semaphores,
    # which splits multi-semaphore waits — TPB instructions carry only one
    # wait slot in hardware — plus move_matmul_waits_to_ldweights etc.
    nc = bacc.Bacc("TRN2", target_bir_lowering=False, debug=False,
                   num_devices=NCORES)
    if "fp32" in mode:
        in_dt = F32
    elif "u16" in mode:
        in_dt = mybir.dt.float16
    else:
        in_dt = mybir.dt.float32r
    if "q16" in mode:
        sq_in_dt = mybir.dt.float16
    elif "bxx" in mode:
        sq_in_dt = mybir.dt.bfloat16
    else:
        sq_in_dt = in_dt
    out_dt = mybir.dt.float16 if "o16" in mode else F32
    xs = nc.dram_tensor("xs", [128, NQ, B], in_dt, kind="ExternalInput")
    wm = nc.dram_tensor("wm", [NQ, 128, 480], in_dt, kind="ExternalInput")
    xxs = nc.dram_tensor("xxs", [NQ, 128, 4, B], sq_in_dt, kind="ExternalInput")
    gs = nc.dram_tensor("gs", [128, NG, 30], sq_in_dt, kind="ExternalInput")
    out = nc.dram_tensor("out", [NQ, 2, 2, 128, 960], out_dt,
                         kind="ExternalOutput")
    with tile.TileContext(nc) as tc:
        _capsule_body(tc, out[:], xs[:], wm[:], xxs[:], gs[:],
                      reps=reps, mode=mode)

    # All ACT functions used here (Copy, Ln, Exp) coexist in the
    # natural_log_exp_and_others table set, but the stock table-load pass
    # assigns each function its *first* containing set, alternating sets and
    # inserting ~2.7us table loads throughout.  Strip our functions from all
    # other sets (keeping positional act_func_set ids intact) so resolution
    # lands on the one set and a single load is emitted.
    import types
    from concourse.hw_specs import get_activation_tables
    from concourse import bacc as _bacc_mod

    _PIN = "natural_log_exp_and_others"
    _FUNCS = {mybir.ActivationFunctionType.Square,
              mybir.ActivationFunctionType.Ln,
              mybir.ActivationFunctionType.Exp,
              mybir.ActivationFunctionType.Copy,
              mybir.ActivationFunctionType.Identity}

    def _one_set_table_loads(self):
        tables = [
            (k, (v if k == _PIN else (v - _FUNCS)))
            for k, v in get_activation_tables(self.m.arch).items()
        ]
        _bacc_mod._bass_rust.insert_act_table_loads(self, tables)

    nc.insert_act_table_loads = types.MethodType(_one_set_table_loads, nc)
    nc.compile()
    return nc


_NC = {}


def _get_nc(reps: int = 1, mode: str = "full"):
    key = (reps, mode)
    if key not in _NC:
        _NC[key] = build_bass(reps, mode)
    return _NC[key]


def _pack_inputs(x: np.ndarray, W: np.ndarray):
    """Build per-core xs [32,48,256], wm [48,32,480], xxs [48,128,256],
    gs [48,128,30]."""
    x = np.ascontiguousarray(x, dtype=np.float32)
    W0 = np.ascontiguousarray(W.reshape(C, R, O, I), dtype=np.float32)

    # x stationaries: [R, I, B] -> rows padded to 32, 4 groups stacked on the
    # 128 partitions (full-width DMA): [cores, 128=(k,row), NQ, B]
    xt = x.transpose(1, 2, 0)                        # [R, I, B]
    xs = np.zeros((NCORES, NG, 32, B), np.float32)
    xs[:, :, :24] = xt.reshape(NCORES, NG, 24, B)
    xs = xs.reshape(NCORES, NQ, 4, 32, B).transpose(0, 2, 3, 1, 4)
    xs = np.ascontiguousarray(xs.reshape(NCORES, 128, NQ, B))

    # W moving blocks, 4 groups stacked on partitions: [cores, NQ, 128, 480]
    Wt = W0.transpose(1, 3, 0, 2)                    # [R, I, C, O]
    Wt = Wt.reshape(NCORES, NG, 3, I, C * O)         # k,g,r,i,co
    wm = np.zeros((NCORES, NG, 32, 3, C * O), np.float32)
    for r in range(3):
        wm[:, :, r * I:(r + 1) * I, r] = Wt[:, :, r]
    wm = np.ascontiguousarray(wm.reshape(NCORES, NQ, 128, 480))

    # xx pair products: [B, R, 36] -> [cores, NQ, 4, (3*36 padded 128), B]
    ii = np.array([p[0] for p in PAIRS])
    jj = np.array([p[1] for p in PAIRS])
    xx = x[:, :, ii] * x[:, :, jj]                   # [B, R, 36]
    xxt = xx.transpose(1, 2, 0)                      # [R, 36, B]
    xxs = np.zeros((NCORES, NG, 128, B), np.float32)
    xxs[:, :, :108] = xxt.reshape(NCORES, NG, 108, B)
    xxs = np.ascontiguousarray(
        xxs.reshape(NCORES, NQ, 4, 128, B).transpose(0, 1, 3, 2, 4))

    # Gram columns: [cores, 48, 128, 30] block-diagonal over the 3 routes
    W64 = W0.astype(np.float64)
    G = np.einsum('croi,croj->crij', W64, W64)       # [C, R, I, I]
    Gsym = G[:, :, ii, jj] * np.where(ii == jj, 1.0, 2.0)   # [C, R, 36]
    Gt = Gsym.transpose(1, 2, 0).astype(np.float32)  # [R, 36, C]
    Gt = Gt.reshape(NCORES, NG, 3, NPAIR, C)
    gs = np.zeros((NCORES, NG, 128, 30), np.float32)
    for r in range(3):
        gs[:, :, r * NPAIR:(r + 1) * NPAIR, r * C:(r + 1) * C] = Gt[:, :, r]
    gs = np.ascontiguousarray(gs.transpose(0, 2, 1, 3))   # [cores, 128, 48, 30]
    return xs, wm, xxs, gs


def _unpack_outputs(results):
    """Per-core out [NQ, 2, 2, 128, 960] -> full [B, C, R, O]."""
    full = np.empty((B, C, R, O), dtype=np.float32)
    for k in range(NCORES):
        ok = np.asarray(results[k]["out"], dtype=np.float32)
        # dims: q, half, kk, p, hb, r, c, o ;
        # route_local = 3*(4q + 2*half + kk) + r ; b = 128*hb + p
        ok = ok.reshape(NQ, 2, 2, 128, 2, 3, C, O)
        fk = ok.transpose(4, 3, 6, 0, 1, 2, 5, 7).reshape(B, C, RL, O)
        full[:, :, k * RL:(k + 1) * RL, :] = fk
    return full


DEFAULT_MODE = "full+o16+u16+q16+d60"


def _cast_packed(packed, mode: str):
    xs, wm, xxs, gs = packed
    if "u16" in mode:
        xs = xs.astype(np.float16)
        wm = wm.astype(np.float16)
    if "q16" in mode:
        xxs = xxs.astype(np.float16)
        gs = gs.astype(np.float16)
    elif "bxx" in mode:
        import ml_dtypes
        xxs = xxs.astype(ml_dtypes.bfloat16)
        gs = gs.astype(ml_dtypes.bfloat16)
    return xs, wm, xxs, gs


def run_packed(packed, reps: int = 1, mode: str = DEFAULT_MODE):
    xs, wm, xxs, gs = _cast_packed(packed, mode)
    nc = _get_nc(reps, mode)
    in_maps = [{"xs": xs[k], "wm": wm[k], "xxs": xxs[k], "gs": gs[k]}
               for k in range(NCORES)]
    return run_bass_kernel_spmd(nc, in_maps, list(range(NCORES)))


def kernel(x: np.ndarray, W: np.ndarray, **_ignored):
    x = np.asarray(x, dtype=np.float32)
    W = np.asarray(W, dtype=np.float32)
    assert x.shape == (B, R, I), x.shape
    packed = _pack_inputs(x, W)
    res = run_packed(packed)
    return _unpack_outputs(res.results)



# revision 32
# speedup vs baseline: 2.2460x; 1.0409x over previous
"""Trainium2 Bass kernel for nn_CapsuleLayer (capsule layer: einsum + squash).

  u_hat = einsum('croi,bri->bcro', W[0], x)   # x:[256,1152,8] W:[1,10,1152,16,8]
  out   = squash(u_hat)                       # squash over last (o) axis

Strategy (8 NeuronCores, routes sharded 144/core, full batch per core):
  - Groups of 3 routes.  Per (group, batch-half) ONE psum bank holds both:
      u-MM:  stationary x^T block [32=(3 routes x 8 in + pad), 128 batch],
             moving block-diagonal W [32, 480] -> psum[:, 0:480]
      sq-MM: stationary xx pair-products [128=(3 x 36 pairs + pad), 128 batch],
             moving block-diagonal sym-Gram cols [128, 30] -> psum[:, 480:510]
    where xx[b,(i,j)] = x_i*x_j (i<=j) and Gsym[(i,j),c] = (2-delta_ij)*G[i,j]
    with G = W_cr^T W_cr, so sq-MM emits sq_norm[b, (r,c)] = ||u||^2 directly.
    The PE therefore replaces both the ACT square pass and the DVE group
    reduce of a conventional squash implementation.
  - squash scale s = sq/((1+sq)*sqrt(sq+1e-9)) = exp(0.5*ln(sq) - ln(1+sq))
    via ACT Ln/Exp (single activation-table set; Rsqrt/Reciprocal on ACT are
    banned for accuracy and DVE reciprocal is slow).
  - DVE does only the final broadcast multiply u * s straight out of PSUM
    into dense SBUF tiles; HWDGE DMAs ship contiguous 240KB blocks.
  - Matmuls run in float32r (single-pass reduced-precision fp32, 4x faster
    than fp32's two half-speed passes; measured end-to-end error ~5e-4
    scale-relative vs the fp32 reference).
"""

import sys

if "/opt/trn_rl_repo" not in sys.path:
    sys.path.insert(0, "/opt/trn_rl_repo")

from contextlib import ExitStack

import numpy as np

import concourse.bacc as bacc
import concourse.bass as bass
import concourse.mybir as mybir
import concourse.tile as tile
from concourse._compat import with_exitstack
from concourse.bass_utils import run_bass_kernel_spmd

# Problem shapes (hardcoded; harness provides full inputs)
B = 256          # batch
R = 1152         # num routes
C = 10           # num capsules
O = 16           # out channels
I = 8            # in channels
NCORES = 8
RL = R // NCORES                 # 144 routes per core
NG = RL // 3                     # 48 groups of 3 routes
NQ = NG // 4                     # 12 quad-blocks of 4 groups (row strips)
NPAIR = 36                       # i<=j pairs of 8 inputs
F32 = mybir.dt.float32
PAIRS = [(i, j) for i in range(I) for j in range(i, I)]


@with_exitstack
def _capsule_body(ctx: ExitStack, tc: "tile.TileContext",
                  out: bass.AP, xs: bass.AP, wm: bass.AP,
                  xxs: bass.AP, gs: bass.AP, reps: int = 1,
                  mode: str = "full"):
    nc = tc.nc

    if "fp32" in mode:
        mm_dt = F32
    elif "u16" in mode:
        # fp16 u-path operands: halves the wm stream and keeps matmul at
        # 1 cycle/row; fp16 mantissa (2^-11) keeps error ~1e-3.
        mm_dt = mybir.dt.float16
    else:
        mm_dt = mybir.dt.float32r
    # Optional: sq-path operands (xx pair products + gram cols) in 16-bit —
    # halves the largest input tensor, enables FWL on the sq-matmul
    # stationary load, and lifts the f32r small-moving (30<256) 4-cyc/row
    # penalty on the sq matmul.
    if "q16" in mode:
        sq_dt = mybir.dt.float16
    elif "bxx" in mode:
        sq_dt = mybir.dt.bfloat16
    else:
        sq_dt = mm_dt
    # fp16 output tiles halve the dominant (23.6MB/core) output stream;
    # host upcasts.  |out| <= 1 so fp16 abs error <= 2^-11.
    out_dt = mybir.dt.float16 if "o16" in mode else F32

    import re

    # '+dN': of the 96 u-mul bank-units per rep, N are multiplied by DVE
    # straight out of PSUM (merged per-half-block instructions); the rest
    # are staged psum->SBUF by ACT and multiplied by Pool (gpsimd), which
    # cannot touch PSUM on TRN2.  Balances DVE/ACT/Pool busy time.
    ndve = re.search(r"\+d(\d+)", mode)
    ndve = int(ndve.group(1)) if ndve else 60
    assert 0 <= ndve <= 96

    singles = ctx.enter_context(tc.tile_pool(name="singles", bufs=1))
    wm_pool = ctx.enter_context(tc.tile_pool(name="wm", bufs=NQ))
    xx_pool = ctx.enter_context(tc.tile_pool(name="xx", bufs=NQ))
    psum_pool = ctx.enter_context(tc.tile_pool(name="psum", bufs=3, space="PSUM"))
    sq_pool = ctx.enter_context(tc.tile_pool(name="sqp", bufs=2, space="PSUM"))
    smalls = ctx.enter_context(tc.tile_pool(name="smalls", bufs=4))
    stage_pool = ctx.enter_context(tc.tile_pool(name="stage", bufs=4))
    out_pool = ctx.enter_context(tc.tile_pool(name="outs", bufs=8))

    # Resident stationaries / gram columns — full-128-partition DMAs (32- or
    # 64-partition transfers run at a fraction of DMA port bandwidth).
    xs_sb = singles.tile([128, NQ * B], mm_dt)
    nc.gpsimd.dma_start(out=xs_sb[:], in_=xs.rearrange("p q b -> p (q b)"))
    gs_sb = singles.tile([128, NG * 30], sq_dt)
    nc.gpsimd.dma_start(out=gs_sb[:], in_=gs.rearrange("p g n -> p (g n)"))
    # wm + xx resident too (5.8MB fp16 total): all input streaming happens
    # once, outside the rep loop; the per-rep loop then only writes output,
    # and the Pool engine is free to take a share of the u-muls.
    wm_sb = []
    xx_sb = []
    for q in range(NQ):
        wt = wm_pool.tile([128, 480], mm_dt)
        nc.gpsimd.dma_start(out=wt[:], in_=wm[q])
        wm_sb.append(wt)
        xt = xx_pool.tile([128, 4 * B], sq_dt)
        nc.gpsimd.dma_start(out=xt[:], in_=xxs[q].rearrange("p k b -> p (k b)"))
        xx_sb.append(xt)

    # Software-pipelined schedule over 48 "blocks" (one route-group g =
    # 4q + 2*half + kk each, both batch halves h).  Block n's u lives in a
    # 2-bank psum tile [128,1024] (bank = h), pool bufs=3 -> pipeline
    # depth 3.  sq is DECOUPLED from the u banks: windows of 4 blocks
    # write their 8 sq results into dedicated [128,240] psum tiles
    # (2 bufs), emitted a full window ahead, so the ACT scale chains run
    # with multi-block lead and the u tiles free as soon as the muls/
    # copies drain them.  Block 0's window+chain come from a one-time
    # prologue; the loop's last window/chain re-computes them for the
    # next rep (same pool slots: allocation counts per rep are multiples
    # of bufs).
    nosq = "nosquash" in mode
    NBL = 4 * NQ                      # 48 blocks; block n == group g=n
    s_tiles = {}

    def sq_window(w, sqt):
        # sq matmuls for blocks 4w..4w+3 into sqt cols [(m, h, 30)].
        for m in range(4):
            g = 4 * w + m
            q, k = g // 4, g % 4
            for h in range(2):
                nc.tensor.matmul(
                    sqt[:, 60 * m + 30 * h: 60 * m + 30 * h + 30],
                    xx_sb[q][:, k * B + h * 128: k * B + h * 128 + 128],
                    gs_sb[:, g * 30: g * 30 + 30], start=True, stop=True,
                    tile_position=(0, 0))

    def chain(sqt):
        # Scale chain for a whole 4-block window [128,240] (one op set per
        # window amortizes the ~185ns ACT access-latency overhead):
        # s = exp(0.5*ln(sq) - ln(1+sq)).
        sq_ap = sqt[:]
        lnsq = smalls.tile([128, 240], F32, tag="lnsq")
        nc.scalar.activation(lnsq[:], sq_ap,
                             mybir.ActivationFunctionType.Ln)
        ln1p = smalls.tile([128, 240], F32, tag="ln1p")
        nc.scalar.activation(ln1p[:], sq_ap,
                             mybir.ActivationFunctionType.Ln, bias=1.0)
        w_t = smalls.tile([128, 240], F32, tag="w")
        nc.vector.scalar_tensor_tensor(
            out=w_t[:], in0=lnsq[:], scalar=0.5, in1=ln1p[:],
            op0=mybir.AluOpType.mult, op1=mybir.AluOpType.subtract)
        s_t = smalls.tile([128, 240], F32, tag="s")
        nc.scalar.activation(s_t[:], w_t[:],
                             mybir.ActivationFunctionType.Exp)
        return s_t

    if not nosq:
        # Prologue: sq window 0 + its chain.
        sqt0 = sq_pool.tile([128, 240], F32, tag="sq")
        sq_window(0, sqt0)
        s_tiles[0] = chain(sqt0)

    if reps > 1:
        # Timing-only variant: run the whole body `reps` times on-device so
        # wall-clock differences cancel host/axon overhead.
        loop_cm = tc.For_i(0, reps, 1)
        ctx.enter_context(loop_cm)

    for n in range(NBL):
        g = n
        q, k = g // 4, g % 4
        if not nosq and n % 4 == 0:
            # sq matmuls + scale chain for the NEXT window (blocks
            # n+4..n+7): a full window of lead before first use.
            w_next = (n // 4 + 1) % NQ
            sq_next = sq_pool.tile([128, 240], F32, tag="sq")
            sq_window(w_next, sq_next)
            s_tiles[w_next] = chain(sq_next)

        ps = psum_pool.tile([128, 1024], F32, tag="ps")
        psb = ps[:].rearrange("p (b w) -> p b w", w=512)
        for h in range(2):
            nc.tensor.matmul(
                ps[:, 512 * h: 512 * h + 480],
                xs_sb[32 * k:32 * k + 32,
                      q * B + h * 128: q * B + h * 128 + 128],
                wm_sb[q][32 * k:32 * k + 32, :], start=True, stop=True,
                tile_position=(32 * k, 0))
        if nosq:
            continue
        s_t = s_tiles[n // 4]
        m = n % 4                      # member index within the window

        # Out tiles are shared by block PAIRS ([128,1920], one DMA per
        # pair); block n writes half e = n%2.  nd of the 2 banks
        # multiplied by DVE straight from psum, the rest staged to SBUF
        # by ACT and multiplied by Pool (which cannot touch PSUM).
        # Bresenham over 96 bank-units hits the '+dN' global DVE share.
        e = n % 2
        if e == 0:
            ot_pair = out_pool.tile([128, 1920], out_dt, tag="ot")
        ot = ot_pair
        nd = ((n + 1) * ndve * 2 // 96) - (n * ndve * 2 // 96)
        nd = max(0, min(2, nd))

        def u_ap(b0, nb):
            return (psb[:, b0:b0 + nb, 0:480]
                    .rearrange("p b (rc v) -> p b rc v", v=O))

        def s_ap(b0, nb):
            return (s_t[:, 60 * m + 30 * b0: 60 * m + 30 * (b0 + nb)]
                    .rearrange("p (b rc) -> p b rc", b=nb)
                    .unsqueeze(3).broadcast_to([128, nb, 30, O]))

        def o_ap(b0, nb):
            return (ot[:, 960 * e + 480 * b0: 960 * e + 480 * (b0 + nb)]
                    .rearrange("p (b rc v) -> p b rc v", b=nb, v=O))

        if nd > 0:
            nc.vector.tensor_mul(o_ap(0, nd), u_ap(0, nd), s_ap(0, nd))
        nb = 2 - nd
        if nb > 0:
            st = stage_pool.tile([128, 960], F32, tag="stg")
            nc.scalar.copy(
                st[:, 0:480 * nb].rearrange("p (b v) -> p b v", b=nb),
                psb[:, nd:2, 0:480])
            st_ap = (st[:, 0:480 * nb]
                     .rearrange("p (b rc v) -> p b rc v", b=nb, v=O))
            nc.gpsimd.tensor_mul(o_ap(nd, nb), st_ap, s_ap(nd, nb))

        if e == 1 and "noout" not in mode:
            # One DMA per pair: a single contiguous 3840B (fp16) segment
            # per partition.
            nc.sync.dma_start(out=out[q, k // 2], in_=ot[:])


def build_bass(reps: int = 1, mode: str = "full"):
    # Bacc (not plain Bass): its compile() runs generate_event_semaphores,
    # which splits multi-semaphore waits — TPB instructions carry only one
    # wait slot in hardware — plus move_matmul_waits_to_ldweights etc.
    nc = bacc.Bacc("TRN2", target_bir_lowering=False, debug=False,
                   num_devices=NCORES)
    if "fp32" in mode:
        in_dt = F32
    elif "u16" in mode:
        in_dt = mybir.dt.float16
    else:
        in_dt = mybir.dt.float32r
    if "q16" in mode:
        sq_in_dt = mybir.dt.float16
    elif "bxx" in mode:
        sq_in_dt = mybir.dt.bfloat16
    else:
        sq_in_dt = in_dt
    out_dt = mybir.dt.float16 if "o16" in mode else F32
    xs = nc.dram_tensor("xs", [128, NQ, B], in_dt, kind="ExternalInput")
    wm = nc.dram_tensor("wm", [NQ, 128, 480], in_dt, kind="ExternalInput")
    xxs = nc.dram_tensor("xxs", [NQ, 128, 4, B], sq_in_dt, kind="ExternalInput")
    gs = nc.dram_tensor("gs", [128, NG, 30], sq_in_dt, kind="ExternalInput")
    out = nc.dram_tensor("out", [NQ, 2, 128, 1920], out_dt,
                         kind="ExternalOutput")
    with tile.TileContext(nc) as tc:
        _capsule_body(tc, out[:], xs[:], wm[:], xxs[:], gs[:],
                      reps=reps, mode=mode)

    # All ACT functions used here (Copy, Ln, Exp) coexist in the
    # natural_log_exp_and_others table set, but the stock table-load pass
    # assigns each function its *first* containing set, alternating sets and
    # inserting ~2.7us table loads throughout.  Strip our functions from all
    # other sets (keeping positional act_func_set ids intact) so resolution
    # lands on the one set and a single load is emitted.
    import types
    from concourse.hw_specs import get_activation_tables
    from concourse import bacc as _bacc_mod

    _PIN = "natural_log_exp_and_others"
    _FUNCS = {mybir.ActivationFunctionType.Square,
              mybir.ActivationFunctionType.Ln,
              mybir.ActivationFunctionType.Exp,
              mybir.ActivationFunctionType.Copy,
              mybir.ActivationFunctionType.Identity}

    def _one_set_table_loads(self):
        tables = [
            (k, (v if k == _PIN else (v - _FUNCS)))
            for k, v in get_activation_tables(self.m.arch).items()
        ]
        _bacc_mod._bass_rust.insert_act_table_loads(self, tables)

    nc.insert_act_table_loads = types.MethodType(_one_set_table_loads, nc)
    nc.compile()
    return nc


_NC = {}


def _get_nc(reps: int = 1, mode: str = "full"):
    key = (reps, mode)
    if key not in _NC:
        _NC[key] = build_bass(reps, mode)
    return _NC[key]


def _pack_inputs(x: np.ndarray, W: np.ndarray):
    """Build per-core xs [32,48,256], wm [48,32,480], xxs [48,128,256],
    gs [48,128,30]."""
    x = np.ascontiguousarray(x, dtype=np.float32)
    W0 = np.ascontiguousarray(W.reshape(C, R, O, I), dtype=np.float32)

    # x stationaries: [R, I, B] -> rows padded to 32, 4 groups stacked on the
    # 128 partitions (full-width DMA): [cores, 128=(k,row), NQ, B]
    xt = x.transpose(1, 2, 0)                        # [R, I, B]
    xs = np.zeros((NCORES, NG, 32, B), np.float32)
    xs[:, :, :24] = xt.reshape(NCORES, NG, 24, B)
    xs = xs.reshape(NCORES, NQ, 4, 32, B).transpose(0, 2, 3, 1, 4)
    xs = np.ascontiguousarray(xs.reshape(NCORES, 128, NQ, B))

    # W moving blocks, 4 groups stacked on partitions: [cores, NQ, 128, 480]
    Wt = W0.transpose(1, 3, 0, 2)                    # [R, I, C, O]
    Wt = Wt.reshape(NCORES, NG, 3, I, C * O)         # k,g,r,i,co
    wm = np.zeros((NCORES, NG, 32, 3, C * O), np.float32)
    for r in range(3):
        wm[:, :, r * I:(r + 1) * I, r] = Wt[:, :, r]
    wm = np.ascontiguousarray(wm.reshape(NCORES, NQ, 128, 480))

    # xx pair products: [B, R, 36] -> [cores, NQ, 4, (3*36 padded 128), B]
    ii = np.array([p[0] for p in PAIRS])
    jj = np.array([p[1] for p in PAIRS])
    xx = x[:, :, ii] * x[:, :, jj]                   # [B, R, 36]
    xxt = xx.transpose(1, 2, 0)                      # [R, 36, B]
    xxs = np.zeros((NCORES, NG, 128, B), np.float32)
    xxs[:, :, :108] = xxt.reshape(NCORES, NG, 108, B)
    xxs = np.ascontiguousarray(
        xxs.reshape(NCORES, NQ, 4, 128, B).transpose(0, 1, 3, 2, 4))

    # Gram columns: [cores, 48, 128, 30] block-diagonal over the 3 routes
    W64 = W0.astype(np.float64)
    G = np.einsum('croi,croj->crij', W64, W64)       # [C, R, I, I]
    Gsym = G[:, :, ii, jj] * np.where(ii == jj, 1.0, 2.0)   # [C, R, 36]
    Gt = Gsym.transpose(1, 2, 0).astype(np.float32)  # [R, 36, C]
    Gt = Gt.reshape(NCORES, NG, 3, NPAIR, C)
    gs = np.zeros((NCORES, NG, 128, 30), np.float32)
    for r in range(3):
        gs[:, :, r * NPAIR:(r + 1) * NPAIR, r * C:(r + 1) * C] = Gt[:, :, r]
    gs = np.ascontiguousarray(gs.transpose(0, 2, 1, 3))   # [cores, 128, 48, 30]
    return xs, wm, xxs, gs


def _unpack_outputs(results):
    """Per-core out [NQ, 2, 128, 1920] -> full [B, C, R, O]."""
    full = np.empty((B, C, R, O), dtype=np.float32)
    for k in range(NCORES):
        ok = np.asarray(results[k]["out"], dtype=np.float32)
        # dims: q, half, p, kk, hb, r, c, o ;
        # route_local = 3*(4q + 2*half + kk) + r ; b = 128*hb + p
        ok = ok.reshape(NQ, 2, 128, 2, 2, 3, C, O)
        fk = ok.transpose(4, 2, 6, 0, 1, 3, 5, 7).reshape(B, C, RL, O)
        full[:, :, k * RL:(k + 1) * RL, :] = fk
    return full


DEFAULT_MODE = "full+o16+u16+q16+d60"


def _cast_packed(packed, mode: str):
    xs, wm, xxs, gs = packed
    if "u16" in mode:
        xs = xs.astype(np.float16)
        wm = wm.astype(np.float16)
    if "q16" in mode:
        xxs = xxs.astype(np.float16)
        gs = gs.astype(np.float16)
    elif "bxx" in mode:
        import ml_dtypes
        xxs = xxs.astype(ml_dtypes.bfloat16)
        gs = gs.astype(ml_dtypes.bfloat16)
    return xs, wm, xxs, gs


def run_packed(packed, reps: int = 1, mode: str = DEFAULT_MODE):
    xs, wm, xxs, gs = _cast_packed(packed, mode)
    nc = _get_nc(reps, mode)
    in_maps = [{"xs": xs[k], "wm": wm[k], "xxs": xxs[k], "gs": gs[k]}
               for k in range(NCORES)]
    return run_bass_kernel_spmd(nc, in_maps, list(range(NCORES)))


def kernel(x: np.ndarray, W: np.ndarray, **_ignored):
    x = np.asarray(x, dtype=np.float32)
    W = np.asarray(W, dtype=np.float32)
    assert x.shape == (B, R, I), x.shape
    packed = _pack_inputs(x, W)
    res = run_packed(packed)
    return _unpack_outputs(res.results)

